# revision 1
# baseline (speedup 1.0000x reference)
"""Trainium2 Bass kernel for the LNN Euler-Lagrange residual.

Math: for a ReLU MLP Lagrangian L(q, qdot) the JAX second-derivative term
d/dt(dL/dqdot) is identically zero (piecewise-linear network), so the
reference output reduces to -dL/dq:

    z1 = x @ W1 + b1          s1 = z1 > 0      a1 = relu(z1)
    z2 = a1 @ W2 + b2         s2 = z2 > 0
    pre1 = s2 @ W2T_eff       (W2T_eff[j,i] = w3[j] * W2[i,j])
    out  = (pre1 * s1) @ (-W1[:32,:].T)

Layout: feature-major (features on partitions, batch streams as matmul
free dim). Host pre-transposes the input shard to [64, B_core]. Two
batch groups are packed on the 128 partitions via host-built 128x128
block-diagonal / anti-diagonal stationary matrices, so every matmul
uses the full PE array with K=128.

z1/z2 matmuls run as float32r (e8m11, full-rate at N=512; inputs and
stationaries are host-pre-rounded so the BIR verifier sees consistent
f32r producers). The backward value path (pure mask x constant) runs in
bf16. Per 512-col block the four PSUM evictions are spread over the two
PSUM-capable engines: ACT does relu and the s2 mask (sigmoid(1e30*z2)
saturates to an exact {0,1} bf16 mask; a searched subset of blocks runs
s2 as is_gt on DVE for load balance); DVE does the fused (a1>0)*pre1
multiply and
the once-per-pair output eviction. Output is stored bf16, two blocks
packed per [128,512] eviction with the store on the Pool SWDGE ring.
Inputs are prefetched 4 blocks ahead; junk matmuls during the DMA
preamble warm the PE clock gate.
"""

import sys

sys.path.insert(0, "/opt/trn_rl_repo")

from contextlib import ExitStack

import numpy as np

B, D, H = 262144, 32, 64
NCORES = 8
BC = B // NCORES          # samples per core
G = BC // 2               # samples per group (2 groups on 128 partitions)
CHUNK = 512               # batch columns per pipeline block (per group)
MMN = 512                 # matmul free-dim (one fp32 PSUM bank)
A_SIG = 1024              # s2 sigmoid cols per 1024 (full sigmoid)

_CACHE = {}


def _round_f32r(a):
    """IEEE fp32 -> e8m11 (float32r): round mantissa to 11 bits (RNE)."""
    u = np.ascontiguousarray(a, np.float32).view(np.uint32)
    lsb = (u >> np.uint32(12)) & np.uint32(1)
    u2 = (u + np.uint32(0x7FF) + lsb) & np.uint32(0xFFFFF000)
    return u2.view(np.float32)


SIG_DVE_SET = frozenset({1, 4, 7, 8, 20, 27})  # best from pattern search


def _build(bc, chunk, a_sig=None, bufs=None, copy_dve=True, sig_dve_every=5,
           w_resid=False, relu_dve_every=0, sig_dve_set=SIG_DVE_SET):
    import concourse.bass as bass
    import concourse.tile as tile
    from concourse import bacc, mybir

    if a_sig is None:
        a_sig = A_SIG * chunk // 1024
    if a_sig > chunk:
        a_sig = chunk
    BUFS = {"xs": 12, "a1": 12, "s2": 8, "t1": 8, "ot": 6,
            "pz1": 2, "pz2": 2, "pp1": 2, "po": 2, "pf": 4}
    if bufs:
        BUFS.update(bufs)

    f32 = mybir.dt.float32
    f32r = mybir.dt.float32r
    bf16 = mybir.dt.bfloat16
    Relu = mybir.ActivationFunctionType.Relu
    Copy = mybir.ActivationFunctionType.Copy
    Sigmoid = mybir.ActivationFunctionType.Sigmoid
    is_gt = mybir.AluOpType.is_gt
    mult = mybir.AluOpType.mult
    bypass = mybir.AluOpType.bypass
    add_op = mybir.AluOpType.add
    max_op = mybir.AluOpType.max

    g = bc // 2
    widths = [chunk] * (g // chunk)
    starts = [0]
    for w_ in widths[:-1]:
        starts.append(starts[-1] + w_)
    nchunks = len(widths)
    # output pairs: consecutive equal-width blocks
    pair_off = [0]
    for p_ in range(nchunks // 2 - 1):
        pair_off.append(pair_off[-1] + widths[2 * p_])
    nmm = 1

    nc = bacc.Bacc("TRN2", target_bir_lowering=False, debug=False)

    # xT rows: p = grp*64 + f (group grp's feature f); cols: samples in group
    # (f32r streams are host-pre-rounded to e8m11)
    xT = nc.dram_tensor("xT", [128, g], f32r, kind="ExternalInput").ap()
    S1 = nc.dram_tensor("S1", [128, 128], f32r, kind="ExternalInput").ap()
    S2 = nc.dram_tensor("S2", [128, 128], f32r, kind="ExternalInput").ap()
    if w_resid:
        S1E = nc.dram_tensor("S1E", [128, 128], f32r, kind="ExternalInput").ap()
        S2E = nc.dram_tensor("S2E", [128, 128], f32r, kind="ExternalInput").ap()
    S3 = nc.dram_tensor("S3", [128, 128], bf16, kind="ExternalInput").ap()
    S4 = nc.dram_tensor("S4", [128, 64], bf16, kind="ExternalInput").ap()
    BIASES = nc.dram_tensor("BIASES", [128, 2], f32, kind="ExternalInput").ap()
    # outT rows (blocks of 32): A-even / B-even / A-odd / B-odd block outputs;
    # cols: pair*chunk + col
    outT = nc.dram_tensor("outT", [128, g // 2], bf16, kind="ExternalOutput").ap()

    with tile.TileContext(nc) as tc, ExitStack() as ctx:
        wp = ctx.enter_context(tc.tile_pool(name="w", bufs=1))
        xs_p = ctx.enter_context(tc.tile_pool(name="xs", bufs=BUFS["xs"]))
        a1_p = ctx.enter_context(tc.tile_pool(name="a1", bufs=BUFS["a1"]))
        s2_p = ctx.enter_context(tc.tile_pool(name="s2", bufs=BUFS["s2"]))
        t1_p = ctx.enter_context(tc.tile_pool(name="t1", bufs=BUFS["t1"]))
        ot_p = ctx.enter_context(tc.tile_pool(name="ot", bufs=BUFS["ot"]))
        pz1 = ctx.enter_context(tc.tile_pool(name="pz1", bufs=BUFS["pz1"], space="PSUM"))
        pz2 = ctx.enter_context(tc.tile_pool(name="pz2", bufs=BUFS["pz2"], space="PSUM"))
        pp1 = ctx.enter_context(tc.tile_pool(name="pp1", bufs=BUFS["pp1"], space="PSUM"))
        pout = ctx.enter_context(tc.tile_pool(name="po", bufs=BUFS["po"], space="PSUM"))

        s1w_t = wp.tile([128, 128], f32r, tag="s1w")
        s1e_t = wp.tile([128, 128], f32r, tag="s1e")
        s2e_t = wp.tile([128, 128], f32r, tag="s2e")
        s2w_t = wp.tile([128, 128], f32r, tag="s2w")
        s3w_t = wp.tile([128, 128], bf16, tag="s3w")
        s4w_t = wp.tile([128, 64], bf16, tag="s4w")
        bia_t = wp.tile([128, 2], f32, tag="bia")
        # S1 first on the HWDGE (SP) ring: L1(0) needs it. bia/S3/S4 go via
        # SWDGE on the idle Pool ring; S2 follows the first two xs loads.
        nc.sync.dma_start(out=s1w_t[:], in_=S1)
        nc.gpsimd.dma_start(out=bia_t[:], in_=BIASES)
        nc.gpsimd.dma_start(out=s3w_t[:], in_=S3)
        nc.gpsimd.dma_start(out=s4w_t[:], in_=S4)
        if w_resid:
            nc.gpsimd.dma_start(out=s1e_t[:], in_=S1E)
            nc.gpsimd.dma_start(out=s2e_t[:], in_=S2E)
        s1w, s2w, s3w, s4w, bia = (s1w_t[:], s2w_t[:], s3w_t[:], s4w_t[:],
                                   bia_t[:])

        # Rolling 4-stage software pipeline: at step c the PE stream gets
        # L1(c), L4(c-3), L3(c-2), L2(c-1); elementwise ops of one block
        # overlap matmuls of neighbouring blocks. PSUM: 4 pools x 2 banks.
        # Input loads are prefetched 4 blocks ahead so L1 never waits on
        # the DMA ring.
        a1s = {}
        s2ms = {}
        t1s = {}
        outps = {}
        xss = {}

        def load_xs(i, eng=None):
            w = widths[i]
            xs = xs_p.tile([128, chunk], f32r, tag="xs", name="xs")
            (eng or nc.sync).dma_start(
                out=xs[:, 0:w], in_=xT[:, starts[i]:starts[i] + w])
            xss[i] = xs

        for i in range(min(BUFS["pf"], nchunks)):
            load_xs(i)
            if i == 1:
                nc.sync.dma_start(out=s2w_t[:], in_=S2)

        # PE warm-up during the DMA preamble: junk matmuls on a zeroed
        # tile (no DMA dependency) keep the HAM activity window busy from
        # t~0 so real matmuls start at the full 2.4 GHz clock.
        wsrc = wp.tile([128, 128], f32, tag="wsrc")
        nc.vector.memset(wsrc[:], 0.0)
        warm = pz1.tile([128, chunk], f32, tag="pz1", name="warm")
        for _ in range(8):
            nc.tensor.matmul(warm[:, 0:128], lhsT=wsrc[:].bitcast(f32r),
                             rhs=wsrc[:].bitcast(f32r), start=True, stop=True)

        for c in range(nchunks + 3):
            # Issue order is tuned per engine queue so every queue head has
            # the earliest-resolving deps:
            #   PE:  L1(c), L4(c-3), L3(c-2), L2(c-1)
            #   ACT: relu(c), sig(c-1)
            #   DVE: copy(pair(c-3)), t1(c-2), isgt(c-1)
            # L1(c) only waits on prefetched xs and relu(c-2) (PSUM bufs=2),
            # so the PE never idles behind the freshest relu.
            if c < nchunks:
                w = widths[c]
                xs = xss.pop(c)
                # L1: z1 = S1.T @ xs (f32r, A->p0:64, B->p64:128)
                z1p = pz1.tile([128, chunk], f32, tag="pz1", name="z1p")
                nc.tensor.matmul(z1p[:, 0:w], lhsT=s1w, rhs=xs[:, 0:w],
                                 start=True, stop=not w_resid)
                if w_resid:
                    nc.tensor.matmul(z1p[:, 0:w], lhsT=s1e_t[:],
                                     rhs=xs[:, 0:w], start=False, stop=True)
                # a1 = relu(z1 + b1), evicts z1p; periodically on DVE to
                # balance the engines
                a1 = a1_p.tile([128, chunk], f32r, tag="a1", name="a1")
                if relu_dve_every and c % relu_dve_every == 1:
                    nc.vector.tensor_scalar(out=a1[:, 0:w], in0=z1p[:, 0:w],
                                            scalar1=bia[:, 0:1], scalar2=0.0,
                                            op0=add_op, op1=max_op)
                else:
                    nc.scalar.activation(out=a1[:, 0:w], in_=z1p[:, 0:w],
                                         func=Relu, bias=bia[:, 0:1],
                                         scale=1.0)
                a1s[c] = a1

            if 0 <= c - 3 < nchunks:
                i = c - 3
                w = widths[i]
                par = i % 2
                pair = i // 2
                # L4: out = S4.T @ t1 -> [64,w]; even block lands on
                # partitions 0:64, odd on 64:128 of the same PSUM tile
                if par == 0:
                    outps[pair] = pout.tile([128, chunk], f32, tag="po",
                                            name="outp")
                op_ = outps[pair]
                t1i = t1s.pop(i)
                nc.tensor.matmul(op_[64 * par:64 * (par + 1), 0:w],
                                 lhsT=s4w, rhs=t1i[:, 0:w],
                                 start=True, stop=True)
                if par == 1:
                    ot = ot_p.tile([128, chunk], bf16, tag="ot", name="ot")
                    # drain: the last two pairs evict + store via the ACT
                    # ring (idle once its relu/sig stream ends) so the DVE
                    # tail only holds the final t1s
                    tail = 2 * pair >= nchunks - 4
                    if copy_dve and not tail:
                        nc.vector.tensor_scalar(out=ot[:, 0:w],
                                                in0=op_[:, 0:w],
                                                scalar1=0.0, scalar2=None,
                                                op0=bypass)
                    else:
                        nc.scalar.activation(out=ot[:, 0:w], in_=op_[:, 0:w],
                                             func=Copy)
                    po_ = pair_off[pair]
                    if tail:
                        nc.scalar.dma_start(out=outT[:, po_:po_ + w],
                                            in_=ot[:, 0:w])
                    else:
                        # SWDGE on the idle Pool ring: keeps output stores
                        # off the shared HWDGE and off the ACT sequencer
                        nc.gpsimd.dma_start(out=outT[:, po_:po_ + w],
                                            in_=ot[:, 0:w])
                    del outps[pair]

            if 0 <= c - 2 < nchunks:
                i = c - 2
                w = widths[i]
                # L3: pre1 = S3.T @ s2 (bf16, groups swap back)
                p1p = pp1.tile([128, chunk], f32, tag="pp1", name="p1p")
                nc.tensor.matmul(p1p[:, 0:w], lhsT=s3w, rhs=s2ms[i][:, 0:w],
                                 start=True, stop=True)
                # t1 = (a1 > 0) * pre1 -> bf16, fused mask+mult (DVE),
                # evicts p1p
                t1 = t1_p.tile([128, chunk], bf16, tag="t1", name="t1")
                nc.vector.scalar_tensor_tensor(
                    out=t1[:, 0:w], in0=a1s.pop(i)[:, 0:w].bitcast(f32),
                    scalar=0.0, in1=p1p[:, 0:w], op0=is_gt, op1=mult)
                t1s[i] = t1
                del s2ms[i]

            if 0 <= c - 1 < nchunks:
                i = c - 1
                w = widths[i]
                # L2: z2 = S2.T @ a1 (f32r, groups swap halves)
                z2p = pz2.tile([128, chunk], f32, tag="pz2", name="z2p")
                nc.tensor.matmul(z2p[:, 0:w], lhsT=s2w, rhs=a1s[i][:, 0:w],
                                 start=True, stop=not w_resid)
                if w_resid:
                    nc.tensor.matmul(z2p[:, 0:w], lhsT=s2e_t[:],
                                     rhs=a1s[i][:, 0:w], start=False,
                                     stop=True)
                # s2 = (z2 > -b2) as bf16 mask; evicts z2p. ACT:
                # sigmoid(1e30 * (z2 + b2)) saturates to exact {0,1} in
                # bf16 for any |z2+b2| > ~3e-29.
                s2m = s2_p.tile([128, chunk], bf16, tag="s2", name="s2m")
                if sig_dve_set is not None:
                    on_dve = i in sig_dve_set
                else:
                    on_dve = (sig_dve_every
                              and i % sig_dve_every == sig_dve_every - 1
                              and i < nchunks - 4)
                asg = 0 if on_dve else min(a_sig, w)

                if asg > 0:
                    nc.scalar.activation(out=s2m[:, 0:asg],
                                         in_=z2p[:, 0:asg],
                                         func=Sigmoid, bias=bia[:, 1:2],
                                         scale=1e30)
                if asg < w:
                    nc.vector.tensor_scalar(out=s2m[:, asg:w],
                                            in0=z2p[:, asg:w],
                                            scalar1=bia[:, 1:2], scalar2=None,
                                            op0=is_gt)
                s2ms[i] = s2m

            if BUFS["pf"] <= c + BUFS["pf"] < nchunks:
                load_xs(c + BUFS["pf"])

    nc.compile()
    return nc


def _get_nc(bc=BC, chunk=CHUNK, **kw):
    key = (bc, chunk, str(kw))
    if key not in _CACHE:
        _CACHE[key] = _build(bc, chunk, **kw)
    return _CACHE[key]


def _host_prep(W1, b1, W2, b2, W3, b3):
    import ml_dtypes

    w3 = np.asarray(W3)[:, 0].astype(np.float32)
    W1 = np.asarray(W1, np.float32)
    W2 = np.asarray(W2, np.float32)
    b1 = np.asarray(b1, np.float32)
    b2 = np.asarray(b2, np.float32)

    S1 = np.zeros((128, 128), np.float32)
    S1[:64, :64] = W1
    S1[64:, 64:] = W1
    S2 = np.zeros((128, 128), np.float32)
    S2[:64, 64:] = W2
    S2[64:, :64] = W2
    S3s = (W2 * w3[None, :]).T  # [j, i] = w3[j] * W2[i, j]
    S3 = np.zeros((128, 128), np.float32)
    S3[64:, :64] = S3s  # A: s2 at p64:128 -> pre1 at p0:64
    S3[:64, 64:] = S3s  # B: s2 at p0:64   -> pre1 at p64:128
    S4s = -(W1[:32, :].T)  # [64, 32]
    S4 = np.zeros((128, 64), np.float32)
    S4[:64, :32] = S4s   # A: t1 p0:64   -> out p0:32 (+64 for odd blocks)
    S4[64:, 32:] = S4s   # B: t1 p64:128 -> out p32:64 (+64 for odd blocks)
    BIASES = np.zeros((128, 2), np.float32)
    BIASES[:, 0] = np.concatenate([b1, b1])
    BIASES[:, 1] = -np.concatenate([b2, b2])
    return {
        "S1": _round_f32r(S1),
        "S2": _round_f32r(S2),
        "S3": S3.astype(ml_dtypes.bfloat16),
        "S4": S4.astype(ml_dtypes.bfloat16),
        "BIASES": BIASES,
    }


def kernel(inputs, W1, b1, W2, b2, W3, b3):
    from concourse.bass_utils import run_bass_kernel_spmd

    x = np.ascontiguousarray(np.asarray(inputs, np.float32))
    consts = _host_prep(W1, b1, W2, b2, W3, b3)

    in_maps = []
    for k in range(NCORES):
        xc = x[k * BC:(k + 1) * BC]          # [BC, 64]
        # rows p = grp*64+f: group A samples [0,G) then group B [G,2G)
        xTk = _round_f32r(np.ascontiguousarray(
            np.concatenate([xc[:G].T, xc[G:].T], axis=0)))  # [128, G]
        in_maps.append({"xT": xTk, **consts})

    nc = _get_nc()
    res = run_bass_kernel_spmd(nc, in_maps, core_ids=list(range(NCORES)),
                               trace=False)
    # block table must mirror _build
    widths = [CHUNK] * (G // CHUNK)
    starts = np.cumsum([0] + widths[:-1])
    pair_off = np.cumsum([0] + [widths[2 * p] for p in
                                range(len(widths) // 2 - 1)])
    outs = []
    for k in range(NCORES):
        oT = np.asarray(res.results[k]["outT"]).astype(np.float32)
        a = np.empty((G, 32), np.float32)
        b = np.empty((G, 32), np.float32)
        for p in range(len(widths) // 2):
            w = widths[2 * p]
            blk = oT[:, pair_off[p]:pair_off[p] + w]
            # rows: 4 groups of 32 = A-even / B-even / A-odd / B-odd
            se, so = starts[2 * p], starts[2 * p + 1]
            a[se:se + w] = blk[0:32].T
            b[se:se + w] = blk[32:64].T
            a[so:so + w] = blk[64:96].T
            b[so:so + w] = blk[96:128].T
        outs.append(a)
        outs.append(b)
    out = np.concatenate(outs, axis=0).astype(np.float32)
    kernel._last_result = res
    return out



# revision 4
# speedup vs baseline: 1.1317x; 1.1317x over previous
"""Trainium2 Bass kernel for the LNN Euler-Lagrange residual.

Math: for a ReLU MLP Lagrangian L(q, qdot) the JAX second-derivative term
d/dt(dL/dqdot) is identically zero (piecewise-linear network), so the
reference output reduces to -dL/dq:

    z1 = x @ W1 + b1          s1 = z1 > 0      a1 = relu(z1)
    z2 = a1 @ W2 + b2         s2 = z2 > 0
    pre1 = s2 @ W2T_eff       (W2T_eff[j,i] = w3[j] * W2[i,j])
    out  = (pre1 * s1) @ (-W1[:32,:].T)

Layout: feature-major (features on partitions, batch streams as matmul
free dim). Host pre-transposes the input shard to [64, B_core]. Two
batch groups are packed on the 128 partitions via host-built 128x128
block-diagonal / anti-diagonal stationary matrices, so every matmul
uses the full PE array with K=128.

Five-stage software pipeline (L1@c, L2@c+1, L3@c+3, L4@c+4) with one
PSUM eviction per engine per step: ACT does relu (z1->a1, f32r), Pool
does the s2 mask (is_gt, fp16), DVE does the fused (a1>0)*pre1 multiply
(fp16). The per-pair output eviction alternates ACT/DVE. Inputs ship as
fp16 (halves HBM traffic; masks lose ~1 bit vs f32r), stationaries
S1/S2 stay f32r, the value path (S3/S4/t1/out) is fp16. The whole input
is DMA'd up front in graduated chunks; outputs batch 2 pairs per store.
"""

import sys

sys.path.insert(0, "/opt/trn_rl_repo")

from contextlib import ExitStack

import numpy as np

B, D, H = 262144, 32, 64
NCORES = 8
BC = B // NCORES          # samples per core
G = BC // 2               # samples per group (2 groups on 128 partitions)
CHUNK = 512               # batch columns per pipeline block (per group)

_CACHE = {}


def _round_f32r(a):
    """IEEE fp32 -> e8m11 (float32r): round mantissa to 11 bits (RNE)."""
    u = np.ascontiguousarray(a, np.float32).view(np.uint32)
    lsb = (u >> np.uint32(12)) & np.uint32(1)
    u2 = (u + np.uint32(0x7FF) + lsb) & np.uint32(0xFFFFF000)
    return u2.view(np.float32)


# input DMA chunk widths (columns); graduated so block 0 starts early
XPLAN = [512, 512, 1024, 1024, 2048, 2048, 2048, 2048, 2048, 2048, 1024]


def _build(bc, chunk, x_dt="fp16", warm=14, bufs=None):
    import concourse.bass as bass
    import concourse.tile as tile
    from concourse import bacc, mybir

    f32 = mybir.dt.float32
    f32r = mybir.dt.float32r
    fp16 = mybir.dt.float16
    bf16 = mybir.dt.bfloat16
    Relu = mybir.ActivationFunctionType.Relu
    Copy = mybir.ActivationFunctionType.Copy
    is_gt = mybir.AluOpType.is_gt
    mult = mybir.AluOpType.mult
    bypass = mybir.AluOpType.bypass

    xdt = fp16 if x_dt == "fp16" else f32r

    g = bc // 2
    nb = g // chunk
    assert sum(XPLAN) == g
    # block -> (chunk index, offset within chunk)
    blk_chunk = []
    ci, coff, cstart = 0, 0, 0
    for b_ in range(nb):
        if coff >= XPLAN[ci]:
            ci += 1
            cstart += coff
            coff = 0
        blk_chunk.append((ci, coff))
        coff += chunk
    BUFS = {"a1": 6, "s2": 6, "t1": 5, "ot": 4}
    if bufs:
        BUFS.update(bufs)

    nc = bacc.Bacc("TRN2", target_bir_lowering=False, debug=False)

    # xT rows: p = grp*64 + f (group grp's feature f); cols: samples in group
    xT = nc.dram_tensor("xT", [128, g], xdt, kind="ExternalInput").ap()
    S1 = nc.dram_tensor("S1", [128, 128], f32r, kind="ExternalInput").ap()
    S2 = nc.dram_tensor("S2", [128, 128], f32r, kind="ExternalInput").ap()
    S3 = nc.dram_tensor("S3", [128, 128], fp16, kind="ExternalInput").ap()
    S4 = nc.dram_tensor("S4", [128, 64], fp16, kind="ExternalInput").ap()
    BIASES = nc.dram_tensor("BIASES", [128, 2], f32, kind="ExternalInput").ap()
    # outT rows (blocks of 32): A-even / B-even / A-odd / B-odd block outputs;
    # cols: pair*chunk + col
    outT = nc.dram_tensor("outT", [128, g // 2], fp16, kind="ExternalOutput").ap()

    with tile.TileContext(nc) as tc, ExitStack() as ctx:
        wp = ctx.enter_context(tc.tile_pool(name="w", bufs=1))
        xs_p = ctx.enter_context(tc.tile_pool(name="xs", bufs=1))
        a1_p = ctx.enter_context(tc.tile_pool(name="a1", bufs=BUFS["a1"]))
        s2_p = ctx.enter_context(tc.tile_pool(name="s2", bufs=BUFS["s2"]))
        t1_p = ctx.enter_context(tc.tile_pool(name="t1", bufs=BUFS["t1"]))
        ot_p = ctx.enter_context(tc.tile_pool(name="ot", bufs=BUFS["ot"]))
        pz1 = ctx.enter_context(tc.tile_pool(name="pz1", bufs=2, space="PSUM"))
        pz2 = ctx.enter_context(tc.tile_pool(name="pz2", bufs=2, space="PSUM"))
        pp1 = ctx.enter_context(tc.tile_pool(name="pp1", bufs=2, space="PSUM"))
        pout = ctx.enter_context(tc.tile_pool(name="po", bufs=2, space="PSUM"))

        s1w_t = wp.tile([128, 128], f32r, tag="s1w")
        s2w_t = wp.tile([128, 128], f32r, tag="s2w")
        s3w_t = wp.tile([128, 128], fp16, tag="s3w")
        s4w_t = wp.tile([128, 64], fp16, tag="s4w")
        bia_t = wp.tile([128, 2], f32, tag="bia")

        # whole input up front: chunk 0 on the ACT queue (parallel with S1 on
        # SP) so block 0's data and stationary land together ~2.5us in; the
        # rest stream on SP with no interleaved waits.
        xs_tiles = []
        off = 0
        for k, w_ in enumerate(XPLAN):
            xs_tiles.append(xs_p.tile([128, w_], xdt, tag=f"xs{k}",
                                      name=f"xs{k}"))
        nc.sync.dma_start(out=s1w_t[:], in_=S1)
        nc.scalar.dma_start(out=xs_tiles[0][:], in_=xT[:, 0:XPLAN[0]])
        nc.scalar.dma_start(out=bia_t[:], in_=BIASES)
        nc.scalar.dma_start(out=s3w_t[:], in_=S3)
        off = XPLAN[0]
        nc.sync.dma_start(out=xs_tiles[1][:], in_=xT[:, off:off + XPLAN[1]])
        nc.sync.dma_start(out=s2w_t[:], in_=S2)
        nc.sync.dma_start(out=s4w_t[:], in_=S4)
        off += XPLAN[1]
        for k in range(2, len(XPLAN)):
            nc.sync.dma_start(out=xs_tiles[k][:], in_=xT[:, off:off + XPLAN[k]])
            off += XPLAN[k]
        s1w, s2w, s3w, s4w, bia = (s1w_t[:], s2w_t[:], s3w_t[:], s4w_t[:],
                                   bia_t[:])

        # PE warm-up: junk bf16 matmuls (no data deps) advance the clock-gate
        # ramp so real matmuls hit 2.4 GHz as soon as the first chunk lands.
        wsrc = wp.tile([128, 128], bf16, tag="wsrc")
        nc.vector.memset(wsrc[:], 0.0)
        warm_t = pz1.tile([128, chunk], f32, tag="pz1", name="warm")
        for _ in range(warm):
            nc.tensor.matmul(warm_t[:, 0:128], lhsT=wsrc[:], rhs=wsrc[:],
                             start=True, stop=True)

        a1s = {}
        s2ms = {}
        t1s = {}
        ots = {}

        # Five-stage pipeline; per step c the PE stream is
        #   L1(c), L2(c-1), L3(c-3), L4(c-4)
        # and each eviction engine gets exactly one [128,chunk] op:
        #   ACT relu(c) / Pool isgt(c-1) / DVE t1(c-3), with the pair output
        #   eviction alternating ACT/DVE.
        for c in range(nb + 4):
            if c < nb:
                ci, coff = blk_chunk[c]
                xs = xs_tiles[ci][:, coff:coff + chunk]
                z1p = pz1.tile([128, chunk], f32, tag="pz1", name="z1p")
                nc.tensor.matmul(z1p[:], lhsT=s1w, rhs=xs,
                                 start=True, stop=True)
                a1 = a1_p.tile([128, chunk], f32r, tag="a1", name="a1")
                nc.scalar.activation(out=a1[:], in_=z1p[:], func=Relu,
                                     bias=bia[:, 0:1], scale=1.0)
                a1s[c] = a1

            if 0 <= c - 1 < nb:
                i = c - 1
                z2p = pz2.tile([128, chunk], f32, tag="pz2", name="z2p")
                nc.tensor.matmul(z2p[:], lhsT=s2w, rhs=a1s[i][:],
                                 start=True, stop=True)
                s2m = s2_p.tile([128, chunk], fp16, tag="s2", name="s2m")
                nc.gpsimd.tensor_scalar(out=s2m[:], in0=z2p[:],
                                        scalar1=bia[:, 1:2], scalar2=None,
                                        op0=is_gt)
                s2ms[i] = s2m

            if 0 <= c - 3 < nb:
                i = c - 3
                p1p = pp1.tile([128, chunk], f32, tag="pp1", name="p1p")
                nc.tensor.matmul(p1p[:], lhsT=s3w, rhs=s2ms.pop(i)[:],
                                 start=True, stop=True)
                t1 = t1_p.tile([128, chunk], fp16, tag="t1", name="t1")
                nc.vector.scalar_tensor_tensor(
                    out=t1[:], in0=a1s.pop(i)[:].bitcast(f32),
                    scalar=0.0, in1=p1p[:], op0=is_gt, op1=mult)
                t1s[i] = t1

            if 0 <= c - 4 < nb:
                i = c - 4
                par = i % 2
                pair = i // 2
                if par == 0:
                    ots[pair] = pout.tile([128, chunk], f32, tag="po",
                                          name="outp")
                op_ = ots[pair]
                nc.tensor.matmul(op_[64 * par:64 * (par + 1), :],
                                 lhsT=s4w, rhs=t1s.pop(i)[:],
                                 start=True, stop=True)
                if par == 1:
                    if pair % 2 == 0:
                        ots["sb"] = ot_p.tile([128, 2 * chunk], fp16,
                                              tag="ot", name="ot")
                    ot = ots["sb"]
                    sl = ot[:, (pair % 2) * chunk:(pair % 2) * chunk + chunk]
                    if pair % 2 == 0:
                        nc.scalar.activation(out=sl, in_=op_[:], func=Copy)
                    else:
                        nc.vector.tensor_scalar(out=sl, in0=op_[:],
                                                scalar1=0.0, scalar2=None,
                                                op0=bypass)
                    del ots[pair]
                    if pair % 2 == 1:
                        po_ = (pair - 1) * chunk
                        nc.sync.dma_start(out=outT[:, po_:po_ + 2 * chunk],
                                          in_=ot[:, :])

    nc.compile()
    return nc


def _get_nc(bc=BC, chunk=CHUNK, **kw):
    key = (bc, chunk, str(kw))
    if key not in _CACHE:
        _CACHE[key] = _build(bc, chunk, **kw)
    return _CACHE[key]


def _host_prep(W1, b1, W2, b2, W3, b3):
    w3 = np.asarray(W3)[:, 0].astype(np.float32)
    W1 = np.asarray(W1, np.float32)
    W2 = np.asarray(W2, np.float32)
    b1 = np.asarray(b1, np.float32)
    b2 = np.asarray(b2, np.float32)

    S1 = np.zeros((128, 128), np.float32)
    S1[:64, :64] = W1
    S1[64:, 64:] = W1
    S2 = np.zeros((128, 128), np.float32)
    S2[:64, 64:] = W2
    S2[64:, :64] = W2
    S3s = (W2 * w3[None, :]).T  # [j, i] = w3[j] * W2[i, j]
    S3 = np.zeros((128, 128), np.float32)
    S3[64:, :64] = S3s  # A: s2 at p64:128 -> pre1 at p0:64
    S3[:64, 64:] = S3s  # B: s2 at p0:64   -> pre1 at p64:128
    S4s = -(W1[:32, :].T)  # [64, 32]
    S4 = np.zeros((128, 64), np.float32)
    S4[:64, :32] = S4s   # A: t1 p0:64   -> out p0:32 (+64 for odd blocks)
    S4[64:, 32:] = S4s   # B: t1 p64:128 -> out p32:64 (+64 for odd blocks)
    BIASES = np.zeros((128, 2), np.float32)
    BIASES[:, 0] = np.concatenate([b1, b1])
    BIASES[:, 1] = -np.concatenate([b2, b2])
    return {
        "S1": _round_f32r(S1),
        "S2": _round_f32r(S2),
        "S3": S3.astype(np.float16),
        "S4": S4.astype(np.float16),
        "BIASES": BIASES,
    }


def kernel(inputs, W1, b1, W2, b2, W3, b3):
    from concourse.bass_utils import run_bass_kernel_spmd

    x = np.ascontiguousarray(np.asarray(inputs, np.float32))
    consts = _host_prep(W1, b1, W2, b2, W3, b3)

    in_maps = []
    for k in range(NCORES):
        xc = x[k * BC:(k + 1) * BC]          # [BC, 64]
        # rows p = grp*64+f: group A samples [0,G) then group B [G,2G)
        xTk = np.ascontiguousarray(
            np.concatenate([xc[:G].T, xc[G:].T], axis=0)).astype(np.float16)
        in_maps.append({"xT": xTk, **consts})

    nc = _get_nc()
    res = run_bass_kernel_spmd(nc, in_maps, core_ids=list(range(NCORES)),
                               trace=False)
    outs = []
    for k in range(NCORES):
        oT = np.asarray(res.results[k]["outT"]).astype(np.float32)
        a = np.empty((G, 32), np.float32)
        b = np.empty((G, 32), np.float32)
        for p in range(G // (2 * CHUNK)):
            blk = oT[:, p * CHUNK:(p + 1) * CHUNK]
            se, so = 2 * p * CHUNK, (2 * p + 1) * CHUNK
            # rows: 4 groups of 32 = A-even / B-even / A-odd / B-odd
            a[se:se + CHUNK] = blk[0:32].T
            b[se:se + CHUNK] = blk[32:64].T
            a[so:so + CHUNK] = blk[64:96].T
            b[so:so + CHUNK] = blk[96:128].T
        outs.append(a)
        outs.append(b)
    out = np.concatenate(outs, axis=0).astype(np.float32)
    kernel._last_result = res
    return out


# revision 8
# speedup vs baseline: 1.1657x; 1.0300x over previous
"""Trainium2 Bass kernel for the LNN Euler-Lagrange residual.

Math: for a ReLU MLP Lagrangian L(q, qdot) the JAX second-derivative term
d/dt(dL/dqdot) is identically zero (piecewise-linear network), so the
reference output reduces to -dL/dq:

    z1 = x @ W1 + b1          s1 = z1 > 0      a1 = relu(z1)
    z2 = a1 @ W2 + b2         s2 = z2 > 0
    pre1 = s2 @ W2T_eff       (W2T_eff[j,i] = w3[j] * W2[i,j])
    out  = (pre1 * s1) @ (-W1[:32,:].T)

Layout: feature-major (features on partitions, batch streams as matmul
free dim). Host pre-transposes the input shard to [64, B_core]. Two
batch groups are packed on the 128 partitions via host-built 128x128
block-diagonal / anti-diagonal stationary matrices, so every matmul
uses the full PE array with K=128.

Five-stage software pipeline (L1@c, L2@c+1, L3@c+3, L4@c+4) with one
PSUM eviction per engine per step: ACT does relu (z1->a1, f32r), Pool
does the s2 mask (is_gt, fp16), DVE does the fused (a1>0)*pre1 multiply
(fp16). The per-pair output eviction alternates ACT/DVE. Inputs ship as
fp16 (halves HBM traffic; masks lose ~1 bit vs f32r), stationaries
S1/S2 stay f32r, the value path (S3/S4/t1/out) is fp16. The whole input
is DMA'd up front in graduated chunks; outputs batch 2 pairs per store.
"""

import sys

sys.path.insert(0, "/opt/trn_rl_repo")

from contextlib import ExitStack

import numpy as np

B, D, H = 262144, 32, 64
NCORES = 8
BC = B // NCORES          # samples per core
G = BC // 2               # samples per group (2 groups on 128 partitions)
CHUNK = 512               # batch columns per pipeline block (per group)

_CACHE = {}


def _round_f32r(a):
    """IEEE fp32 -> e8m11 (float32r): round mantissa to 11 bits (RNE)."""
    u = np.ascontiguousarray(a, np.float32).view(np.uint32)
    lsb = (u >> np.uint32(12)) & np.uint32(1)
    u2 = (u + np.uint32(0x7FF) + lsb) & np.uint32(0xFFFFF000)
    return u2.view(np.float32)


# input DMA chunk widths (columns); graduated so block 0 starts early
XPLAN = [512, 512, 1024, 1024, 2048, 2048, 2048, 2048, 2048, 2048, 1024]


def _build(bc, chunk, x_dt="fp16", warm=14, bufs=None):
    import concourse.bass as bass
    import concourse.tile as tile
    from concourse import bacc, mybir

    f32 = mybir.dt.float32
    f32r = mybir.dt.float32r
    fp16 = mybir.dt.float16
    bf16 = mybir.dt.bfloat16
    Relu = mybir.ActivationFunctionType.Relu
    Copy = mybir.ActivationFunctionType.Copy
    is_gt = mybir.AluOpType.is_gt
    mult = mybir.AluOpType.mult
    bypass = mybir.AluOpType.bypass

    xdt = fp16 if x_dt == "fp16" else f32r

    g = bc // 2
    nb = g // chunk
    assert sum(XPLAN) == g
    # block -> (chunk index, offset within chunk)
    blk_chunk = []
    ci, coff, cstart = 0, 0, 0
    for b_ in range(nb):
        if coff >= XPLAN[ci]:
            ci += 1
            cstart += coff
            coff = 0
        blk_chunk.append((ci, coff))
        coff += chunk
    BUFS = {"a1": 6, "s2": 6, "t1": 5, "ot": 4}
    if bufs:
        BUFS.update(bufs)

    nc = bacc.Bacc("TRN2", target_bir_lowering=False, debug=False)

    # xT rows: p = grp*64 + f (group grp's feature f); cols: samples in group
    xT = nc.dram_tensor("xT", [128, g], xdt, kind="ExternalInput").ap()
    S1 = nc.dram_tensor("S1", [128, 128], f32r, kind="ExternalInput").ap()
    S2 = nc.dram_tensor("S2", [128, 128], f32r, kind="ExternalInput").ap()
    S3 = nc.dram_tensor("S3", [128, 128], fp16, kind="ExternalInput").ap()
    S4 = nc.dram_tensor("S4", [128, 64], fp16, kind="ExternalInput").ap()
    BIASES = nc.dram_tensor("BIASES", [128, 2], f32, kind="ExternalInput").ap()
    # outT rows (blocks of 32): A-even / B-even / A-odd / B-odd block outputs;
    # cols: pair*chunk + col
    outT = nc.dram_tensor("outT", [128, g // 2], fp16, kind="ExternalOutput").ap()

    with tile.TileContext(nc) as tc, ExitStack() as ctx:
        wp = ctx.enter_context(tc.tile_pool(name="w", bufs=1))
        xs_p = ctx.enter_context(tc.tile_pool(name="xs", bufs=1))
        a1_p = ctx.enter_context(tc.tile_pool(name="a1", bufs=BUFS["a1"]))
        s2_p = ctx.enter_context(tc.tile_pool(name="s2", bufs=BUFS["s2"]))
        t1_p = ctx.enter_context(tc.tile_pool(name="t1", bufs=BUFS["t1"]))
        ot_p = ctx.enter_context(tc.tile_pool(name="ot", bufs=BUFS["ot"]))
        pz1 = ctx.enter_context(tc.tile_pool(name="pz1", bufs=2, space="PSUM"))
        pz2 = ctx.enter_context(tc.tile_pool(name="pz2", bufs=2, space="PSUM"))
        pp1 = ctx.enter_context(tc.tile_pool(name="pp1", bufs=2, space="PSUM"))
        pout = ctx.enter_context(tc.tile_pool(name="po", bufs=2, space="PSUM"))

        s1w_t = wp.tile([128, 128], f32r, tag="s1w")
        s2w_t = wp.tile([128, 128], f32r, tag="s2w")
        s3w_t = wp.tile([128, 128], fp16, tag="s3w")
        s4w_t = wp.tile([128, 64], fp16, tag="s4w")
        bia_t = wp.tile([128, 2], f32, tag="bia")

        # whole input up front: chunk 0 on the ACT queue (parallel with S1 on
        # SP) so block 0's data and stationary land together ~2.5us in; the
        # rest stream on SP with no interleaved waits.
        xs_tiles = []
        off = 0
        for k, w_ in enumerate(XPLAN):
            xs_tiles.append(xs_p.tile([128, w_], xdt, tag=f"xs{k}",
                                      name=f"xs{k}"))
        xoff = [0]
        for w_ in XPLAN[:-1]:
            xoff.append(xoff[-1] + w_)
        # memsets first so the dummy activation (ACT table preload) can start
        # as soon as the scalar queue has issued the first input chunk
        wsrc = wp.tile([128, 128], bf16, tag="wsrc")
        dum = wp.tile([128, 4], f32, tag="dum")
        nc.gpsimd.memset(wsrc[:], 0.0)
        nc.gpsimd.memset(dum[:], 0.0)

        nc.sync.dma_start(out=s1w_t[:], in_=S1)
        nc.scalar.dma_start(out=xs_tiles[0][:],
                            in_=xT[:, xoff[0]:xoff[0] + XPLAN[0]])
        # dummy activation absorbs the one-time LoadActFuncSet (~1.3us) off
        # the critical path before relu(0) needs the ACT engine
        nc.scalar.activation(out=dum[:], in_=dum[:], func=Relu, scale=1.0)
        nc.scalar.dma_start(out=bia_t[:], in_=BIASES)
        nc.scalar.dma_start(out=s3w_t[:], in_=S3)
        nc.sync.dma_start(out=xs_tiles[1][:],
                          in_=xT[:, xoff[1]:xoff[1] + XPLAN[1]])
        nc.sync.dma_start(out=xs_tiles[2][:],
                          in_=xT[:, xoff[2]:xoff[2] + XPLAN[2]])
        nc.sync.dma_start(out=s2w_t[:], in_=S2)
        nc.sync.dma_start(out=xs_tiles[3][:],
                          in_=xT[:, xoff[3]:xoff[3] + XPLAN[3]])
        nc.sync.dma_start(out=s4w_t[:], in_=S4)
        for k in range(4, len(XPLAN)):
            nc.sync.dma_start(out=xs_tiles[k][:],
                              in_=xT[:, xoff[k]:xoff[k] + XPLAN[k]])
        s1w, s2w, s3w, s4w, bia = (s1w_t[:], s2w_t[:], s3w_t[:], s4w_t[:],
                                   bia_t[:])

        # PE warm-up: junk bf16 matmuls (no data deps) advance the clock-gate
        # ramp so real matmuls hit 2.4 GHz as soon as the first chunk lands.
        warm_t = pz1.tile([128, chunk], f32, tag="pz1", name="warm")
        for _ in range(warm):
            nc.tensor.matmul(warm_t[:, 0:128], lhsT=wsrc[:], rhs=wsrc[:],
                             start=True, stop=True)

        a1s = {}
        s2ms = {}
        t1s = {}
        ots = {}
        e4b_q = []
        dma_q = []
        npairs = nb // 2
        E4A = 320  # output-eviction columns on ACT; rest on DVE (staggered)

        # Five-stage pipeline; per step c the PE stream is
        #   L1(c), L2(c-1), L3(c-3), L4(c-4)
        # and each eviction engine gets exactly one [128,chunk] op:
        #   ACT relu(c) / Pool isgt(c-1) / DVE t1(c-3). The pair output
        #   eviction is split ACT/DVE (DVE half staggered one step) so no
        #   engine spikes past the PE block budget.
        for c in range(nb + 4):
            # staggered DVE half of the previous pair's output eviction
            while e4b_q:
                op_, sl = e4b_q.pop()
                nc.vector.tensor_scalar(out=sl, in0=op_[:, E4A:chunk],
                                        scalar1=0.0, scalar2=None, op0=bypass)
            while dma_q:
                po_, ot_ = dma_q.pop()
                nc.sync.dma_start(out=outT[:, po_:po_ + 2 * chunk],
                                  in_=ot_[:, :])
            if c < nb:
                ci, coff = blk_chunk[c]
                xs = xs_tiles[ci][:, coff:coff + chunk]
                z1p = pz1.tile([128, chunk], f32, tag="pz1", name="z1p")
                nc.tensor.matmul(z1p[:], lhsT=s1w, rhs=xs,
                                 start=True, stop=True)
                a1 = a1_p.tile([128, chunk], f32r, tag="a1", name="a1")
                nc.scalar.activation(out=a1[:], in_=z1p[:], func=Relu,
                                     bias=bia[:, 0:1], scale=1.0)
                a1s[c] = a1

            if 0 <= c - 1 < nb:
                i = c - 1
                z2p = pz2.tile([128, chunk], f32, tag="pz2", name="z2p")
                nc.tensor.matmul(z2p[:], lhsT=s2w, rhs=a1s[i][:],
                                 start=True, stop=True)
                s2m = s2_p.tile([128, chunk], fp16, tag="s2", name="s2m")
                nc.gpsimd.tensor_scalar(out=s2m[:], in0=z2p[:],
                                        scalar1=bia[:, 1:2], scalar2=None,
                                        op0=is_gt)
                s2ms[i] = s2m

            if 0 <= c - 3 < nb:
                i = c - 3
                p1p = pp1.tile([128, chunk], f32, tag="pp1", name="p1p")
                nc.tensor.matmul(p1p[:], lhsT=s3w, rhs=s2ms.pop(i)[:],
                                 start=True, stop=True)
                t1 = t1_p.tile([128, chunk], fp16, tag="t1", name="t1")
                nc.vector.scalar_tensor_tensor(
                    out=t1[:], in0=a1s.pop(i)[:].bitcast(f32),
                    scalar=0.0, in1=p1p[:], op0=is_gt, op1=mult)
                t1s[i] = t1

            if 0 <= c - 4 < nb:
                i = c - 4
                par = i % 2
                pair = i // 2
                if par == 0:
                    ots[pair] = pout.tile([128, chunk], f32, tag="po",
                                          name="outp")
                op_ = ots[pair]
                nc.tensor.matmul(op_[64 * par:64 * (par + 1), :],
                                 lhsT=s4w, rhs=t1s.pop(i)[:],
                                 start=True, stop=True)
                if par == 1:
                    tail = pair >= npairs - 2
                    if pair % 2 == 0:
                        ots["sb"] = ot_p.tile([128, 2 * chunk], fp16,
                                              tag="ot", name="ot")
                    ot = ots["sb"]
                    base = (pair % 2) * chunk
                    if tail:
                        # drain: full eviction on ACT (its relu stream has
                        # ended) and an immediate per-pair store
                        nc.scalar.activation(out=ot[:, base:base + chunk],
                                             in_=op_[:], func=Copy)
                        nc.scalar.dma_start(
                            out=outT[:, pair * chunk:(pair + 1) * chunk],
                            in_=ot[:, base:base + chunk])
                    else:
                        nc.scalar.activation(out=ot[:, base:base + E4A],
                                             in_=op_[:, 0:E4A], func=Copy)
                        e4b_q.append((op_, ot[:, base + E4A:base + chunk]))
                        if pair % 2 == 1:
                            dma_q.append(((pair - 1) * chunk, ot))
                    del ots[pair]

    nc.compile()
    return nc


def _get_nc(bc=BC, chunk=CHUNK, **kw):
    key = (bc, chunk, str(kw))
    if key not in _CACHE:
        _CACHE[key] = _build(bc, chunk, **kw)
    return _CACHE[key]


def _host_prep(W1, b1, W2, b2, W3, b3):
    w3 = np.asarray(W3)[:, 0].astype(np.float32)
    W1 = np.asarray(W1, np.float32)
    W2 = np.asarray(W2, np.float32)
    b1 = np.asarray(b1, np.float32)
    b2 = np.asarray(b2, np.float32)

    S1 = np.zeros((128, 128), np.float32)
    S1[:64, :64] = W1
    S1[64:, 64:] = W1
    S2 = np.zeros((128, 128), np.float32)
    S2[:64, 64:] = W2
    S2[64:, :64] = W2
    S3s = (W2 * w3[None, :]).T  # [j, i] = w3[j] * W2[i, j]
    S3 = np.zeros((128, 128), np.float32)
    S3[64:, :64] = S3s  # A: s2 at p64:128 -> pre1 at p0:64
    S3[:64, 64:] = S3s  # B: s2 at p0:64   -> pre1 at p64:128
    S4s = -(W1[:32, :].T)  # [64, 32]
    S4 = np.zeros((128, 64), np.float32)
    S4[:64, :32] = S4s   # A: t1 p0:64   -> out p0:32 (+64 for odd blocks)
    S4[64:, 32:] = S4s   # B: t1 p64:128 -> out p32:64 (+64 for odd blocks)
    BIASES = np.zeros((128, 2), np.float32)
    BIASES[:, 0] = np.concatenate([b1, b1])
    BIASES[:, 1] = -np.concatenate([b2, b2])
    return {
        "S1": _round_f32r(S1),
        "S2": _round_f32r(S2),
        "S3": S3.astype(np.float16),
        "S4": S4.astype(np.float16),
        "BIASES": BIASES,
    }


def kernel(inputs, W1, b1, W2, b2, W3, b3):
    from concourse.bass_utils import run_bass_kernel_spmd

    x = np.ascontiguousarray(np.asarray(inputs, np.float32))
    consts = _host_prep(W1, b1, W2, b2, W3, b3)

    in_maps = []
    for k in range(NCORES):
        xc = x[k * BC:(k + 1) * BC]          # [BC, 64]
        # rows p = grp*64+f: group A samples [0,G) then group B [G,2G)
        xTk = np.ascontiguousarray(
            np.concatenate([xc[:G].T, xc[G:].T], axis=0)).astype(np.float16)
        in_maps.append({"xT": xTk, **consts})

    nc = _get_nc()
    res = run_bass_kernel_spmd(nc, in_maps, core_ids=list(range(NCORES)),
                               trace=False)
    outs = []
    for k in range(NCORES):
        oT = np.asarray(res.results[k]["outT"]).astype(np.float32)
        a = np.empty((G, 32), np.float32)
        b = np.empty((G, 32), np.float32)
        for p in range(G // (2 * CHUNK)):
            blk = oT[:, p * CHUNK:(p + 1) * CHUNK]
            se, so = 2 * p * CHUNK, (2 * p + 1) * CHUNK
            # rows: 4 groups of 32 = A-even / B-even / A-odd / B-odd
            a[se:se + CHUNK] = blk[0:32].T
            b[se:se + CHUNK] = blk[32:64].T
            a[so:so + CHUNK] = blk[64:96].T
            b[so:so + CHUNK] = blk[96:128].T
        outs.append(a)
        outs.append(b)
    out = np.concatenate(outs, axis=0).astype(np.float32)
    kernel._last_result = res
    return out
